# revision 10
# baseline (speedup 1.0000x reference)
"""Trainium2 Bass kernel for EnhancedTripletLoss (hard-mining triplet loss).

Strategy (8 NeuronCores, SPMD, no collectives):
  * Rows (anchors) are sharded BY CLASS: core c handles all anchors of class c
    (8 classes == 8 cores), padded to a uniform slab height Mc.
  * Columns (candidates) are permuted into 8 contiguous class blocks, each
    padded to a multiple of 128 with DUPLICATE columns of the same class
    (duplicates can only tie, never change a max/min).
  * Per core, matmuls compute g[a, j] = -2*<e_a, e_j> + ||e_j||^2 into PSUM,
    with the core's OWN class block sign-negated in the moving operand.  A
    single free-dim min-reduction per block then yields:
        own block:    min(-g) = -(max over positives of (d2 - sqa))
        other blocks: min( g) =  (min over that block's negatives of (d2-sqa))
    so one pass over the distance matrix produces both the hardest positive
    (argmax) and hardest negative (argmin) statistics.  ||e_a||^2 (constant
    per row) is folded in after the reduction.
  * fp32 matmul on TRN2 runs ~4-5x slower than bf16 (HI/LO decomposition),
    so the contraction uses SPLIT-PRECISION bf16: the -2*e_a operand is
    decomposed into NTERMS bf16 terms (hi, residual), the e_j moving operand
    keeps only its bf16 hi part, and ||e_j||^2 rides as a K=3 chunk of three
    bf16 terms (exact to ~1e-6).  PSUM accumulates in fp32.  Measured loss
    error vs the fp32 reference: ~1e-5 (NTERMS=1) / ~4e-6 (NTERMS=2).
  * The torch F.pairwise_distance eps=1e-6 in the reference perturbs the
    recomputed distances by a relative ~3e-8 (< fp32 ulp) for this data, so
    distances are taken directly from the Gram-trick d2 values.
  * Validity (anchor has >=1 positive and >=1 negative) depends only on class
    counts and is computed host-side from the labels; invalid/padding anchors
    are masked by a 0/1 input.
  * Each core writes per-partition partial sums [128, 2] (loss numerator,
    valid count); the host does the final tiny sum + divide.
"""

import numpy as np
import ml_dtypes

P = 128          # SBUF partitions
D = 256          # embedding dim (fixed by the problem)
NCLS = 8         # number of classes == number of cores
NCORES = 8
MARGIN = 0.3
BIGM = 1.0e30    # block-exclusion additive mask (applied to [128,8] stats only)
NTERMS = 1       # bf16 terms for the -2*e_a stationary operand (1 or 2)
SQTERMS = 3      # bf16 terms for the ||e_j||^2 channel

BF16 = ml_dtypes.bfloat16


def _build_program(Mc, Ws, nterms=NTERMS):
    import concourse.tile as tile
    from concourse import bacc, mybir

    f32 = mybir.dt.float32
    bf16 = mybir.dt.bfloat16
    AX = mybir.AxisListType.X
    OP = mybir.AluOpType

    Mt = Mc // P
    N = int(sum(Ws))
    offs = np.concatenate([[0], np.cumsum(Ws)]).astype(int)

    nc = bacc.Bacc("TRN2", target_bir_lowering=False, debug=False)

    # DRAM I/O (per-core tensors; same names/shapes on every core)
    v0 = nc.dram_tensor("v0", [P, N], bf16, kind="ExternalInput")
    v1 = nc.dram_tensor("v1", [P, N], bf16, kind="ExternalInput")
    v2 = nc.dram_tensor("v2", [SQTERMS, N], bf16, kind="ExternalInput")
    uts = [
        nc.dram_tensor(f"u{k}t{t}", [P, Mc], bf16, kind="ExternalInput")
        for t in range(nterms) for k in range(2)
    ]
    sqa = nc.dram_tensor("sqa", [P, Mt], f32, kind="ExternalInput")
    vld = nc.dram_tensor("valid", [P, Mt], f32, kind="ExternalInput")
    pbig = nc.dram_tensor("posbig", [P, NCLS], f32, kind="ExternalInput")
    nbig = nc.dram_tensor("negbig", [P, NCLS], f32, kind="ExternalInput")
    out = nc.dram_tensor("out", [P, 2], f32, kind="ExternalOutput")

    with tile.TileContext(nc) as tc:
        with (
            tc.tile_pool(name="resident", bufs=1) as res,
            tc.tile_pool(name="psum", bufs=2, space="PSUM") as pp,
            tc.tile_pool(name="bmins", bufs=3) as bmp,
            tc.tile_pool(name="epi", bufs=12) as epi,
        ):
            # ---- resident loads -------------------------------------------
            utiles = []
            for i, ut in enumerate(uts):
                t = res.tile([P, Mc], bf16, tag=f"ut{i}", name=f"ut{i}")
                nc.sync.dma_start(out=t[:], in_=ut[:, :])
                utiles.append(t)
            u2t = res.tile([SQTERMS, Mc], bf16, tag="u2")
            nc.vector.memset(u2t[:], 1.0)

            sqat = res.tile([P, Mt], f32, tag="sqa")
            nc.sync.dma_start(out=sqat[:], in_=sqa[:, :])
            vldt = res.tile([P, Mt], f32, tag="valid")
            nc.sync.dma_start(out=vldt[:], in_=vld[:, :])
            pbigt = res.tile([P, NCLS], f32, tag="posbig")
            nc.sync.dma_start(out=pbigt[:], in_=pbig[:, :])
            nbigt = res.tile([P, NCLS], f32, tag="negbig")
            nc.sync.dma_start(out=nbigt[:], in_=nbig[:, :])

            v0ts, v1ts, v2ts = [], [], []
            for b in range(NCLS):
                W = int(Ws[b])
                o = int(offs[b])
                t0 = res.tile([P, W], bf16, tag=f"v0b{b}", name=f"v0b{b}")
                nc.sync.dma_start(out=t0[:], in_=v0[:, o:o + W])
                t1 = res.tile([P, W], bf16, tag=f"v1b{b}", name=f"v1b{b}")
                nc.sync.dma_start(out=t1[:], in_=v1[:, o:o + W])
                t2 = res.tile([SQTERMS, W], bf16, tag=f"v2b{b}", name=f"v2b{b}")
                nc.sync.dma_start(out=t2[:], in_=v2[:, o:o + W])
                v0ts.append(t0)
                v1ts.append(t1)
                v2ts.append(t2)

            num_sb = res.tile([P, Mt], f32, tag="num")
            out_sb = res.tile([P, 2], f32, tag="out")

            # ---- PE warmup ------------------------------------------------
            # ~4us of dummy matmuls during the DMA phase so the PE's HAM
            # clock-gate reaches 8/8 (2.4 GHz) before the real stream starts.
            wsrc = res.tile([P, 512], bf16, tag="wsrc")
            nc.vector.memset(wsrc[:], 0.0)
            wp = pp.tile([P, 512], f32, tag="warm", name="warm")
            for _ in range(48):
                nc.tensor.matmul(wp[:, :], wsrc[:, 0:P], wsrc[:, :],
                                 start=True, stop=True)

            # ---- main loop ------------------------------------------------
            # Loop order: stationary operand (term) OUTER, N-subtile inner —
            # one LDWEIGHTS per (block, term) instead of per matmul.
            for mt in range(Mt):
                ms = slice(mt * P, (mt + 1) * P)
                bmins = bmp.tile([P, NCLS], f32, tag="bm")
                for b in range(NCLS):
                    W = int(Ws[b])
                    ptile = pp.tile([P, W], f32, tag="pblk", name="pblk")
                    subs = [(i, min(512, W - i)) for i in range(0, W, 512)]
                    stats = []
                    for t in range(nterms):
                        stats.append((utiles[2 * t], v0ts[b]))
                        stats.append((utiles[2 * t + 1], v1ts[b]))
                    stats.append((u2t, v2ts[b]))
                    for ti, (ut, vt) in enumerate(stats):
                        for i, s in subs:
                            cs = slice(i, i + s)
                            nc.tensor.matmul(
                                ptile[:, cs], ut[:, ms], vt[:, cs],
                                start=(ti == 0), stop=(ti == len(stats) - 1),
                            )
                    # reduction: min over the whole block -> bmins[:, b]
                    nc.vector.tensor_reduce(
                        bmins[:, b:b + 1], ptile[:, :], axis=AX, op=OP.min,
                    )

                # ---- epilogue for this anchor tile ------------------------
                t8a = epi.tile([P, NCLS], f32, tag="t8a")
                nc.vector.tensor_add(t8a[:], bmins[:], pbigt[:])
                mown = epi.tile([P, 1], f32, tag="mown")
                nc.vector.tensor_reduce(mown[:], t8a[:], axis=AX, op=OP.min)

                t8b = epi.tile([P, NCLS], f32, tag="t8b")
                nc.vector.tensor_add(t8b[:], bmins[:], nbigt[:])
                mneg = epi.tile([P, 1], f32, tag="mneg")
                nc.vector.tensor_reduce(mneg[:], t8b[:], axis=AX, op=OP.min)

                pd2 = epi.tile([P, 1], f32, tag="pd2")
                nc.vector.tensor_sub(pd2[:], sqat[:, mt:mt + 1], mown[:])
                pd2c = epi.tile([P, 1], f32, tag="pd2c")
                nc.vector.tensor_scalar_max(pd2c[:], pd2[:], 0.0)

                nd2 = epi.tile([P, 1], f32, tag="nd2")
                nc.vector.tensor_add(nd2[:], mneg[:], sqat[:, mt:mt + 1])
                nd2c = epi.tile([P, 1], f32, tag="nd2c")
                nc.vector.tensor_scalar_max(nd2c[:], nd2[:], 0.0)

                pdist = epi.tile([P, 1], f32, tag="pdist")
                nc.scalar.sqrt(pdist[:], pd2c[:])
                ndist = epi.tile([P, 1], f32, tag="ndist")
                nc.scalar.sqrt(ndist[:], nd2c[:])

                per = epi.tile([P, 1], f32, tag="per")
                nc.vector.scalar_tensor_tensor(
                    per[:], in0=pdist[:], scalar=MARGIN, in1=ndist[:],
                    op0=OP.add, op1=OP.subtract,
                )
                perr = epi.tile([P, 1], f32, tag="perr")
                nc.vector.tensor_scalar_max(perr[:], per[:], 0.0)
                nc.vector.tensor_tensor(
                    num_sb[:, mt:mt + 1], perr[:], vldt[:, mt:mt + 1], op=OP.mult,
                )

            nc.vector.tensor_reduce(out_sb[:, 0:1], num_sb[:], axis=AX, op=OP.add)
            nc.vector.tensor_reduce(out_sb[:, 1:2], vldt[:], axis=AX, op=OP.add)
            nc.sync.dma_start(out=out[:, :], in_=out_sb[:])

    nc.compile()
    return nc


def _bf16_terms(x, nterms):
    """Decompose fp32 array into a list of bf16 terms summing to ~x."""
    terms = []
    r = x.astype(np.float32)
    for _ in range(nterms):
        h = r.astype(BF16)
        terms.append(h)
        r = r - h.astype(np.float32)
    return terms


def _prepare_inputs(emb, lab, nterms=NTERMS):
    """Host-side shard/layout prep.  Returns (in_maps, meta)."""
    B = emb.shape[0]
    assert emb.shape[1] == D
    counts = np.bincount(lab, minlength=NCLS).astype(int)
    assert counts.sum() == B

    order = np.argsort(lab, kind="stable")
    cstart = np.concatenate([[0], np.cumsum(counts)]).astype(int)

    Ws = np.maximum(P, ((counts + P - 1) // P) * P).astype(int)
    Mc = int(Ws.max())
    Mt = Mc // P
    N = int(Ws.sum())

    sq = np.einsum("ij,ij->i", emb, emb, dtype=np.float32)  # ||e||^2, fp32

    # column index per block (padded with duplicates)
    colidx = np.empty(N, dtype=np.int64)
    offs = np.concatenate([[0], np.cumsum(Ws)]).astype(int)
    for b in range(NCLS):
        idx = order[cstart[b]:cstart[b + 1]]
        if counts[b] == 0:
            idx = order[0:1]  # arbitrary real point; ties only
        pad = np.full(Ws[b] - len(idx), idx[0], dtype=np.int64)
        colidx[offs[b]:offs[b + 1]] = np.concatenate([idx, pad])

    Vg = np.ascontiguousarray(emb[colidx].T).astype(BF16)   # [256, N] bf16 hi
    sq_terms = _bf16_terms(sq, SQTERMS)
    sqf_t = np.stack([t[colidx] for t in sq_terms])          # [SQTERMS, N] bf16

    # stationary -2*e terms, shared layout across cores (sliced per core)
    u_full = _bf16_terms(-2.0 * emb, nterms)                 # list of [B, 256]

    in_maps = []
    for c in range(NCLS):
        # anchors: class-c rows padded to Mc
        aidx = order[cstart[c]:cstart[c + 1]]
        if counts[c] == 0:
            aidx = order[0:1]
        npad = Mc - len(aidx)
        pad = np.full(npad, aidx[0], dtype=np.int64)
        aidx_p = np.concatenate([aidx, pad])

        real = np.zeros(Mc, dtype=np.float32)
        real[: min(len(aidx), Mc)] = 1.0
        cls_valid = 1.0 if (2 <= counts[c] <= B - 1) else 0.0
        valid = (real * cls_valid).reshape(Mt, P).T.copy()  # [128, Mt]

        sqa_t = sq[aidx_p].reshape(Mt, P).T.copy()          # [128, Mt]

        s = np.ones(N, dtype=np.float32)
        s[offs[c]:offs[c + 1]] = -1.0
        sb = s.astype(BF16)  # +-1 exact

        posbig = np.full((P, NCLS), BIGM, dtype=np.float32)
        posbig[:, c] = 0.0
        negbig = np.zeros((P, NCLS), dtype=np.float32)
        negbig[:, c] = BIGM

        im = {
            "v0": np.ascontiguousarray(Vg[0:128] * sb),
            "v1": np.ascontiguousarray(Vg[128:256] * sb),
            "v2": np.ascontiguousarray(sqf_t * sb),
            "sqa": sqa_t,
            "valid": valid,
            "posbig": posbig,
            "negbig": negbig,
        }
        for t in range(nterms):
            ut = u_full[t][aidx_p]                           # [Mc, 256] bf16
            im[f"u0t{t}"] = np.ascontiguousarray(ut[:, 0:128].T)
            im[f"u1t{t}"] = np.ascontiguousarray(ut[:, 128:256].T)
        in_maps.append(im)

    meta = dict(Mc=Mc, Ws=tuple(int(w) for w in Ws), Mt=Mt, N=N)
    return in_maps, meta


_PROGRAM_CACHE = {}


def _get_program(Mc, Ws):
    key = (Mc, Ws, NTERMS)
    if key not in _PROGRAM_CACHE:
        _PROGRAM_CACHE[key] = _build_program(Mc, Ws, NTERMS)
    return _PROGRAM_CACHE[key]


def _combine(results):
    num = 0.0
    den = 0.0
    for r in results:
        o = np.asarray(r["out"], dtype=np.float64)
        num += o[:, 0].sum()
        den += o[:, 1].sum()
    return np.float32(num / max(den, 1.0))


def _setup_trace_hook():
    """Register the axon NTFF profile hook if the image lacks antenv.axon_hooks."""
    import sys
    import types
    try:
        from antenv.axon_hooks import get_axon_ntff_profile_hook  # noqa: F401
        return
    except ImportError:
        pass
    import antenv
    from trn_agent_boot.trn_boot import _ntff_profile_via_ctypes

    mod = types.ModuleType("antenv.axon_hooks")
    state = {"h": None}
    mod.set_axon_ntff_profile_hook = lambda h: state.__setitem__("h", h)
    mod.get_axon_ntff_profile_hook = lambda: state["h"]
    sys.modules["antenv.axon_hooks"] = mod
    antenv.axon_hooks = mod
    mod.set_axon_ntff_profile_hook(
        _ntff_profile_via_ctypes("/opt/axon/libaxon_pjrt.so")
    )


def kernel(embeddings, labels, _trace=False):
    emb = np.ascontiguousarray(np.asarray(embeddings, dtype=np.float32))
    lab = np.asarray(labels).astype(np.int64).ravel()

    in_maps, meta = _prepare_inputs(emb, lab)
    nc = _get_program(meta["Mc"], meta["Ws"])

    from concourse.bass_utils import run_bass_kernel_spmd

    if _trace:
        _setup_trace_hook()
        import concourse.bass_utils as _bu
        _bu.upload_artifacts = lambda tmpdir: tmpdir  # skip remote upload

    res = run_bass_kernel_spmd(
        nc, in_maps, core_ids=list(range(NCORES)), trace=bool(_trace),
    )
    loss = _combine(res.results)
    if _trace:
        return loss, res
    return loss
